# revision 14
# baseline (speedup 1.0000x reference)
"""Trainium2 Bass kernel for MHA with KV cache.

Problem (hardcoded):
    x:      [2, 2048, 1280] f32
    past_K: [2, 20, 2048, 64] f32
    past_V: [2, 20, 2048, 64] f32
    w_q/w_k/w_v/W_o: [1280, 1280] f32
Returns (out, K, V) like the reference:
    out = softmax((x@w_q)(x@w_k)^T/8) @ (x@w_v) @ W_o   (per head, no mask)
    K/V = concat(past, new) caches.

Sharding: 8 cores; core c handles batch b=c//4, heads 5*(c%4)..+5
(tensor-parallel over (batch, head)). Values are exchanged with a 4-rank
AllGather per batch group; every core then computes its batch's output
projection (redundant within the group; host picks cores 0 and 4).

Numerics: everything up to the softmax (q/k projections, q@kT scores, row
max, exp) is fp32 — the scores have std ~1.3e3 and exp() amplifies any
absolute error, so tf32/bf16 there is not an option.  Post-softmax
(attention weights, V, A@V, W_o) runs in fp16, which keeps the relative
error of the output around 1e-3 while using full-rate matmuls.

Softmax layout trick: scores are computed as [t_partition, s_free] so that
the DVE row-max (tensor_tensor_reduce, fused with the PSUM->SBUF copy) and
the ACT exp (per-partition bias = -max, accum_out = row sum) work along the
free dim; the fp16 attention matrix is then transposed 128x128-block-wise
on the PE so A@V can contract over s on the partition dim.
"""

import math
from dataclasses import dataclass

import numpy as np

import concourse.bass as bass
import concourse.mybir as mybir
import concourse.tile as tile
from concourse import bacc
from concourse.bass_utils import run_bass_kernel_spmd
from concourse.masks import make_identity

F32 = mybir.dt.float32
F16 = mybir.dt.float16
AX = mybir.AxisListType
ALU = mybir.AluOpType
ACTF = mybir.ActivationFunctionType

NEG_BIG = -3.0e38


@dataclass(frozen=True)
class Cfg:
    d: int = 1280        # model dim (multiple of 128)
    t: int = 2048        # new tokens per batch (multiple of 256)
    dh: int = 64         # head dim (fixed 64)
    hpc: int = 5         # heads per core
    s_past: int = 2048   # past KV length (multiple of 512)
    n_cores: int = 8
    group: int = 4       # cores per batch group (AllGather group size)
    phases: str = "ABC"  # debug: which phases to build

    @property
    def s(self):
        return self.s_past + self.t

    @property
    def nd(self):
        return self.d // 128

    @property
    def nt(self):
        return self.t // 128

    @property
    def ns(self):
        return self.s // 128

    @property
    def npair(self):
        return (self.hpc + 1) // 2

    @property
    def g(self):
        return self.hpc * self.dh  # per-core projected width


def _blocks(total, width):
    """split `total` into blocks of at most `width`."""
    out = []
    o = 0
    while o < total:
        w = min(width, total - o)
        out.append((o, w))
        o += w
    return out


def build_program(cfg: Cfg):
    """Build the (SPMD, identical-per-core) Bass program."""
    nc = bacc.Bacc(
        "TRN2",
        target_bir_lowering=False,
        debug=False,
        enable_asserts=True,
        num_devices=cfg.n_cores,
    )

    d, t, dh, hpc, sp = cfg.d, cfg.t, cfg.dh, cfg.hpc, cfg.s_past
    s, nd, nt, ns, npair, g = cfg.s, cfg.nd, cfg.nt, cfg.ns, cfg.npair, cfg.g
    nsb_past = sp // 512  # past s-blocks (512 wide)
    ns_past = sp // 128   # past s-tiles (128 wide)

    # ---- I/O ----
    xT = nc.dram_tensor("xT", [d, t], F32, kind="ExternalInput").ap()
    wq = nc.dram_tensor("wq", [d, g], F32, kind="ExternalInput").ap()
    wk = nc.dram_tensor("wk", [d, g], F32, kind="ExternalInput").ap()
    wv = nc.dram_tensor("wv", [d, g], F32, kind="ExternalInput").ap()
    pastKT = nc.dram_tensor("pastKT", [hpc, dh, sp], F32, kind="ExternalInput").ap()
    pastV = nc.dram_tensor("pastV", [hpc, sp, dh], F32, kind="ExternalInput").ap()
    dfull = cfg.group * g  # gathered values dim (== d in the real config)
    wo16 = nc.dram_tensor("wo16", [dfull, d], F16, kind="ExternalInput").ap()

    out_full = nc.dram_tensor("out_full", [t, d], F32, kind="ExternalOutput").ap()
    k_new = nc.dram_tensor("k_new", [hpc, t, dh], F32, kind="ExternalOutput").ap()
    v_new = nc.dram_tensor("v_new", [hpc, t, dh], F32, kind="ExternalOutput").ap()

    replica_groups = [
        list(range(gs, gs + cfg.group)) for gs in range(0, cfg.n_cores, cfg.group)
    ]

    with tile.TileContext(nc) as tc:
        with tc.tile_pool(name="persist", bufs=1) as pp:
            id16 = pp.tile([128, 128], F16)
            make_identity(nc, id16)
            id32 = pp.tile([128, 128], F32)
            make_identity(nc, id32)

            # persistent activations
            qT = pp.tile([128, npair, t], F32)    # rows: head (2pr) 0:64 | (2pr+1) 64:128
            kTn = pp.tile([128, npair, t], F32)   # new-K transposed, same layout
            v16 = pp.tile([128, nt, g], F16)      # new-V, [t_local, tt, h*dh]
            valuesT = pp.tile([128, npair, t], F16)
            l_all = pp.tile([128, hpc, nt], F32)
            linv_all = pp.tile([128, hpc, nt], F32)

            # =========== Phase A: projections ===========
            with tc.tile_pool(name="proj_sb", bufs=1) as wpool, \
                 tc.tile_pool(name="proj_xt", bufs=1) as xtp, \
                 tc.tile_pool(name="proj_st", bufs=3) as stp, \
                 tc.tile_pool(name="proj_ps", bufs=1, space="PSUM") as pps:
                wq_sb = wpool.tile([128, nd, g], F32)
                nc.sync.dma_start(wq_sb, wq.rearrange("(nd p) g -> p nd g", p=128))
                wk_sb = wpool.tile([128, nd, g], F32)
                nc.sync.dma_start(wk_sb, wk.rearrange("(nd p) g -> p nd g", p=128))
                wv_sb = wpool.tile([128, nd, g], F32)
                nc.sync.dma_start(wv_sb, wv.rearrange("(nd p) g -> p nd g", p=128))

                th_w = min(1024, t)  # t-half width
                for th0, thw in _blocks(t, th_w):
                    xt = []
                    for di in range(nd):
                        xti = xtp.tile([128, th_w], F32, tag=f"xt{di}")
                        nc.sync.dma_start(
                            xti[:, 0:thw], xT[di * 128:(di + 1) * 128, th0:th0 + thw]
                        )
                        xt.append(xti)
                    # q^T and k^T projections: [g, t] with d contraction
                    for wsb, dest in ((wq_sb, qT), (wk_sb, kTn)):
                        for pr in range(npair):
                            m = 128 if 2 * pr + 1 < hpc else 64
                            for tb0, tbw in _blocks(thw, 512):
                                ps = pps.tile([128, 512], F32, tag="projps", bufs=3)
                                for di in range(nd):
                                    nc.tensor.matmul(
                                        ps[0:m, 0:tbw],
                                        lhsT=wsb[:, di, pr * 128:pr * 128 + m],
                                        rhs=xt[di][:, tb0:tb0 + tbw],
                                        start=(di == 0),
                                        stop=(di == nd - 1),
                                    )
                                nc.any.tensor_copy(
                                    dest[0:m, pr, th0 + tb0:th0 + tb0 + tbw],
                                    ps[0:m, 0:tbw],
                                )
                    # v projection: [t, g] orientation
                    for tb0, tbw in _blocks(thw, 128):
                        tt = (th0 + tb0) // 128
                        ps = pps.tile([128, g], F32, tag="projps_v", bufs=3)
                        for di in range(nd):
                            nc.tensor.matmul(
                                ps,
                                lhsT=xt[di][:, tb0:tb0 + 128],
                                rhs=wv_sb[:, di, :],
                                start=(di == 0),
                                stop=(di == nd - 1),
                            )
                        nc.any.tensor_copy(v16[:, tt, :], ps)
                        vst = stp.tile([128, g], F32, tag="vstage")
                        nc.any.tensor_copy(vst, ps)
                        nc.sync.dma_start(
                            v_new[:, tt * 128:(tt + 1) * 128, :].rearrange(
                                "h t e -> t h e"
                            ),
                            vst.rearrange("p (h e) -> p h e", e=dh),
                        )
                # k_new = transpose(kTn)
                for pr in range(npair):
                    m = 128 if 2 * pr + 1 < hpc else 64
                    hpr = m // 64
                    for tt in range(nt):
                        tp = pps.tile([128, 128], F32, tag="ktrans", bufs=2)
                        nc.tensor.transpose(
                            tp[:, 0:m],
                            in_=kTn[0:m, pr, tt * 128:(tt + 1) * 128],
                            identity=id32[0:m, 0:m],
                        )
                        kst = stp.tile([128, 128], F32, tag="kstage")
                        nc.any.tensor_copy(kst[:, 0:m], tp[:, 0:m])
                        nc.sync.dma_start(
                            k_new[
                                2 * pr:2 * pr + hpr, tt * 128:(tt + 1) * 128, :
                            ].rearrange("h t e -> t h e"),
                            kst[:, 0:m].rearrange("p (h e) -> p h e", e=dh),
                        )

            # =========== Phase B: attention ===========
            if "B" in cfg.phases:
              with tc.tile_pool(name="at_kv", bufs=2) as kvp, \
                 tc.tile_pool(name="at_stage", bufs=1) as sgp, \
                 tc.tile_pool(name="at_attn", bufs=3) as ap_, \
                 tc.tile_pool(name="at_attnT", bufs=1) as atp, \
                 tc.tile_pool(name="at_stats", bufs=8) as stq, \
                 tc.tile_pool(name="at_scps", bufs=2, space="PSUM") as scps, \
                 tc.tile_pool(name="at_tpps", bufs=2, space="PSUM") as tpps, \
                 tc.tile_pool(name="at_pvps", bufs=2, space="PSUM") as pvps:
                for pr in range(npair):
                    hpr = 2 if 2 * pr + 1 < hpc else 1
                    kTp = kvp.tile([128, sp], F32, tag="kTpast")
                    pV = kvp.tile([128, ns_past, 128], F16, tag="pastv")
                    for hi in range(hpr):
                        nc.sync.dma_start(
                            kTp[64 * hi:64 * hi + 64, :], pastKT[2 * pr + hi]
                        )
                        nc.gpsimd.dma_start(
                            pV[:, :, 64 * hi:64 * hi + 64],
                            pastV[2 * pr + hi].rearrange("(st p) e -> p st e", p=128),
                        )

                    attnT = [None, None]
                    ch_w = min(1024, s)
                    chunks = _blocks(s, ch_w)
                    nch = len(chunks)
                    for tt in range(nt):
                        attn_t = []
                        nm_parts = []
                        l_parts = []
                        for hi in range(hpr):
                            attn_t.append(
                                ap_.tile(
                                    [128, s], F16, tag=f"attn{hi}",
                                    name=f"attn{hi}_{pr}_{tt}",
                                )
                            )
                            nm_parts.append(
                                stq.tile(
                                    [128, nch], F32, tag=f"nm{hi}",
                                    name=f"nm{hi}_{pr}_{tt}",
                                )
                            )
                            l_parts.append(
                                stq.tile(
                                    [128, nch], F32, tag=f"lp{hi}",
                                    name=f"lp{hi}_{pr}_{tt}",
                                )
                            )
                        # scores (fp32) chunkwise: per-chunk -max, then exp
                        # with the chunk's own max straight out of PSUM.
                        for ci, (c0, cw) in enumerate(chunks):
                            for hi in range(hpr):
                                base = 64 * hi
                                ps = scps.tile([128, ch_w], F32, tag="scps")
                                for sb0, sbw in _blocks(cw, 512):
                                    sb = (c0 + sb0) // 512
                                    if sb < nsb_past:
                                        rhs = kTp[base:base + 64, sb0 + c0:sb0 + c0 + sbw]
                                    else:
                                        o = c0 + sb0 - sp
                                        rhs = kTn[base:base + 64, pr, o:o + sbw]
                                    nc.tensor.matmul(
                                        ps[:, sb0:sb0 + sbw],
                                        lhsT=qT[base:base + 64, pr, tt * 128:(tt + 1) * 128],
                                        rhs=rhs,
                                        start=True,
                                        stop=True,
                                        tile_position=(base, 0),
                                    )
                                nc.vector.tensor_reduce(
                                    nm_parts[hi][:, ci:ci + 1],
                                    ps[:, 0:cw],
                                    axis=AX.X,
                                    op=ALU.max,
                                    negate=True,
                                )
                                nc.scalar.activation(
                                    attn_t[hi][:, c0:c0 + cw],
                                    ps[:, 0:cw],
                                    ACTF.Exp,
                                    bias=nm_parts[hi][:, ci:ci + 1],
                                    scale=1.0,
                                    accum_out=l_parts[hi][:, ci:ci + 1],
                                )
                        for hi in range(hpr):
                            h = 2 * pr + hi
                            attn = attn_t[hi]
                            # combine chunk stats:
                            #   m_final = -min_c nm_c ;  f_c = exp(m_c - m_final)
                            #   l = sum_c l_c * f_c ;   chunk scale = f_c / l
                            nmmin = stq.tile(
                                [128, 1], F32, tag=f"nmm{hi}",
                                name=f"nmm{hi}_{pr}_{tt}",
                            )
                            nc.vector.tensor_reduce(
                                nmmin, nm_parts[hi], axis=AX.X, op=ALU.min
                            )
                            f_parts = stq.tile(
                                [128, nch], F32, tag=f"fp{hi}",
                                name=f"fp{hi}_{pr}_{tt}",
                            )
                            nc.scalar.activation(
                                f_parts, nm_parts[hi], ACTF.Exp,
                                bias=nmmin, scale=-1.0,
                            )
                            lw = stq.tile(
                                [128, nch], F32, tag=f"lw{hi}",
                                name=f"lw{hi}_{pr}_{tt}",
                            )
                            nc.vector.tensor_mul(lw, l_parts[hi], f_parts)
                            nc.vector.reduce_sum(
                                l_all[:, h, tt:tt + 1], lw, axis=AX.X
                            )
                            nc.vector.reciprocal(
                                linv_all[:, h, tt:tt + 1], l_all[:, h, tt:tt + 1]
                            )
                            sc_parts = stq.tile(
                                [128, nch], F32, tag=f"scp{hi}",
                                name=f"scp{hi}_{pr}_{tt}",
                            )
                            nc.vector.tensor_scalar_mul(
                                sc_parts, f_parts, linv_all[:, h, tt:tt + 1]
                            )
                            for ci, (c0, cw) in enumerate(chunks):
                                nc.vector.tensor_scalar_mul(
                                    attn[:, c0:c0 + cw],
                                    attn[:, c0:c0 + cw],
                                    sc_parts[:, ci:ci + 1],
                                )
                            # transpose attn into [s, t] staging (per tt-pair)
                            if tt % 2 == 0:
                                attnT[hi] = atp.tile(
                                    [128, ns, 256], F16, tag=f"attnT{hi}",
                                    name=f"attnT{hi}_{pr}_{tt}",
                                )
                            aT = attnT[hi]
                            for u in range(ns // 4):
                                tp = tpps.tile([128, 512], F16, tag="tp")
                                for j in range(4):
                                    st = 4 * u + j
                                    nc.tensor.transpose(
                                        tp[:, j * 128:(j + 1) * 128],
                                        in_=attn[:, st * 128:(st + 1) * 128],
                                        identity=id16,
                                    )
                                nc.any.tensor_copy(
                                    aT[:, 4 * u:4 * u + 4, (tt % 2) * 128:(tt % 2) * 128 + 128],
                                    tp.rearrange("p (j c) -> p j c", j=4),
                                )
                        if tt % 2 == 1:
                            # A@V for this t-pair (256 t columns)
                            pv = pvps.tile([128, 256], F32, tag="pv")
                            for st in range(ns):
                                for hi in range(hpr):
                                    if st < ns_past:
                                        lhsT = pV[:, st, 64 * hi:64 * hi + 64]
                                    else:
                                        lhsT = v16[
                                            :, st - ns_past,
                                            (2 * pr + hi) * dh:(2 * pr + hi + 1) * dh,
                                        ]
                                    nc.tensor.matmul(
                                        pv[64 * hi:64 * hi + 64, :],
                                        lhsT=lhsT,
                                        rhs=attnT[hi][:, st, :],
                                        start=(st == 0),
                                        stop=(st == ns - 1),
                                        tile_position=(0, 64 * hi),
                                        skip_group_check=True,
                                    )
                            nc.any.tensor_copy(
                                valuesT[0:64 * hpr, pr, (tt - 1) * 128:(tt + 1) * 128],
                                pv[0:64 * hpr, :],
                            )

            # =========== Phase C: AllGather + output projection ===========
            if "C" in cfg.phases:
              with tc.tile_pool(name="wo_dram", bufs=1, space="DRAM") as drp, \
                 tc.tile_pool(name="wo_sb", bufs=1) as wop, \
                 tc.tile_pool(name="wo_st", bufs=4) as wst, \
                 tc.tile_pool(name="wo_ps", bufs=4, space="PSUM") as wps:
                bounce = drp.tile([g, t], F16)
                ag_out = drp.tile([dfull, t], F16)
                for pr in range(npair):
                    m = 128 if 2 * pr + 1 < hpc else 64
                    nc.sync.dma_start(
                        bounce[pr * 128:pr * 128 + m, :], valuesT[0:m, pr, :]
                    )
                nc.gpsimd.collective_compute(
                    "AllGather",
                    ALU.bypass,
                    ins=[bounce.opt()],
                    outs=[ag_out.opt()],
                    replica_groups=replica_groups,
                )
                df_tiles = _blocks(dfull, 128)
                vt_sb = []
                wo_sb = []
                for di, (d0, dw) in enumerate(df_tiles):
                    vti = wop.tile([128, t], F16, tag=f"vt{di}")
                    nc.sync.dma_start(vti[0:dw, :], ag_out[d0:d0 + dw, :])
                    vt_sb.append(vti)
                    woi = wop.tile([128, d], F16, tag=f"wo{di}")
                    nc.sync.dma_start(woi[0:dw, :], wo16[d0:d0 + dw, :])
                    wo_sb.append(woi)
                for tt in range(nt):
                    for nb0, nbw in _blocks(d, 512):
                        ps = wps.tile([128, 512], F32, tag="wops")
                        for di, (d0, dw) in enumerate(df_tiles):
                            nc.tensor.matmul(
                                ps[:, 0:nbw],
                                lhsT=vt_sb[di][0:dw, tt * 128:(tt + 1) * 128],
                                rhs=wo_sb[di][0:dw, nb0:nb0 + nbw],
                                start=(di == 0),
                                stop=(di == len(df_tiles) - 1),
                            )
                        ost = wst.tile([128, 512], F32, tag="ost")
                        nc.any.tensor_copy(ost[:, 0:nbw], ps[:, 0:nbw])
                        nc.sync.dma_start(
                            out_full[tt * 128:(tt + 1) * 128, nb0:nb0 + nbw],
                            ost[:, 0:nbw],
                        )

    nc.compile()
    return nc


# ---------------- host wrapper ----------------

_CACHE: dict = {}


def _get_program(cfg: Cfg):
    if cfg not in _CACHE:
        _CACHE[cfg] = build_program(cfg)
    return _CACHE[cfg]


def make_in_maps(cfg: Cfg, x, past_K, past_V, w_q, w_k, w_v, W_o):
    """Build the 8 per-core input dicts from full inputs (numpy f32)."""
    hpc, group = cfg.hpc, cfg.group
    wq8 = (w_q / 8.0).astype(np.float32)
    wo16 = np.ascontiguousarray(W_o.astype(np.float16))
    in_maps = []
    for c in range(cfg.n_cores):
        b = c // group
        hs = hpc * (c % group)
        he = hs + hpc
        in_maps.append(
            {
                "xT": np.ascontiguousarray(x[b].T),
                "wq": np.ascontiguousarray(wq8[:, hs * cfg.dh:he * cfg.dh]),
                "wk": np.ascontiguousarray(w_k[:, hs * cfg.dh:he * cfg.dh]),
                "wv": np.ascontiguousarray(w_v[:, hs * cfg.dh:he * cfg.dh]),
                "pastKT": np.ascontiguousarray(
                    past_K[b, hs:he].transpose(0, 2, 1)
                ),
                "pastV": np.ascontiguousarray(past_V[b, hs:he]),
                "wo16": wo16,
            }
        )
    return in_maps


def assemble_outputs(cfg: Cfg, results, past_K, past_V):
    hpc, group = cfg.hpc, cfg.group
    n_b = cfg.n_cores // group
    H = hpc * group
    out = np.empty((n_b, cfg.t, cfg.d), np.float32)
    K = np.empty((n_b, H, cfg.s, cfg.dh), np.float32)
    V = np.empty((n_b, H, cfg.s, cfg.dh), np.float32)
    K[:, :, : cfg.s_past] = past_K
    V[:, :, : cfg.s_past] = past_V
    for c in range(cfg.n_cores):
        b = c // group
        hs = hpc * (c % group)
        he = hs + hpc
        if c % group == 0:
            out[b] = results[c]["out_full"]
        K[b, hs:he, cfg.s_past:] = results[c]["k_new"]
        V[b, hs:he, cfg.s_past:] = results[c]["v_new"]
    return out, K, V


def run(cfg: Cfg, x, past_K, past_V, w_q, w_k, w_v, W_o, **spmd_kwargs):
    nc = _get_program(cfg)
    in_maps = make_in_maps(cfg, x, past_K, past_V, w_q, w_k, w_v, W_o)
    res = run_bass_kernel_spmd(
        nc, in_maps, core_ids=list(range(cfg.n_cores)), **spmd_kwargs
    )
    out, K, V = assemble_outputs(cfg, res.results, past_K, past_V)
    return (out, K, V), res


def kernel(x, past_K, past_V, w_q, w_k, w_v, W_o):
    cfg = Cfg()
    args = [
        np.asarray(a, np.float32) for a in (x, past_K, past_V, w_q, w_k, w_v, W_o)
    ]
    (out, K, V), _ = run(cfg, *args)
    return out, K, V


# revision 15
# speedup vs baseline: 1.1003x; 1.1003x over previous
"""Trainium2 Bass kernel for MHA with KV cache.

Problem (hardcoded in Cfg defaults):
    x: [2, 2048, 1280] f32, past_K/past_V: [2, 20, 2048, 64] f32,
    w_q/w_k/w_v/W_o: [1280, 1280] f32.
Returns (out, K, V) like the reference (softmax((x wq)(x wk)^T/8)(x wv) Wo
per head, plus concatenated KV caches).

Sharding: 8 cores; core c handles batch b=c//4 and heads 5*(c%4)..+5.
Each core computes its heads' attention and a partial output projection
over its 320 value dims; a 4-rank ReduceScatter per batch group sums the
partials and hands each core a 512-row t-slice of the final output.

Numerics: the pre-softmax path needs ~20+ mantissa bits (scores have std
~1.3e3 and exp() amplifies absolute error), so q/k projections are fp32
matmuls, and q@kT runs as a 3-pass fp16 split (q = qhi+qlo, k = khi+klo;
hh + hl + lh accumulated in PSUM fp32 gives ~2^-22 products at full f16
PE rate - true fp32 matmul is 1/4 rate and row-duplex packing does not
overlap for it on HW). Row max / exp are exact per 1024-wide chunk (each
chunk exped with its own max straight out of PSUM; the per-chunk
correction exp(m_c - m_final)/l is folded into the one normalization
tensor_scalar pass). Post-softmax runs in fp16.

Layouts: scores are [t_partition, s_free] so DVE row-max and ACT exp
(bias=-max, accum_out=row sum) work along the free dim; the fp16
attention matrix is PE-transposed 128x128-block-wise into a joint
[s, headA(256)|headB(256)] staging so A@V contracts over s with a single
128-col (FWL) V load and one N=512 matmul per s-tile.
"""

from dataclasses import dataclass

import numpy as np

import concourse.mybir as mybir
import concourse.tile as tile
from concourse import bacc
from concourse.bass_utils import run_bass_kernel_spmd
from concourse.masks import make_identity

F32 = mybir.dt.float32
F16 = mybir.dt.float16
AX = mybir.AxisListType
ALU = mybir.AluOpType
ACTF = mybir.ActivationFunctionType


@dataclass(frozen=True)
class Cfg:
    d: int = 1280        # model dim (multiple of 128)
    t: int = 2048        # new tokens per batch (multiple of 256)
    dh: int = 64         # head dim (fixed 64)
    hpc: int = 5         # heads per core
    s_past: int = 2048   # past KV length (multiple of 512)
    n_cores: int = 8
    group: int = 4       # cores per batch group (collective group size)
    phases: str = "ABC"  # debug: which phases to build

    @property
    def s(self):
        return self.s_past + self.t

    @property
    def nd(self):
        return self.d // 128

    @property
    def nt(self):
        return self.t // 128

    @property
    def ns(self):
        return self.s // 128

    @property
    def npair(self):
        return (self.hpc + 1) // 2

    @property
    def g(self):
        return self.hpc * self.dh  # per-core projected width

    @property
    def tshard(self):
        return self.t // self.group


def _blocks(total, width):
    out = []
    o = 0
    while o < total:
        w = min(width, total - o)
        out.append((o, w))
        o += w
    return out


def build_program(cfg: Cfg):
    """Build the (SPMD, identical-per-core) Bass program."""
    nc = bacc.Bacc(
        "TRN2",
        target_bir_lowering=False,
        debug=False,
        enable_asserts=True,
        num_devices=cfg.n_cores,
    )

    d, t, dh, hpc, sp = cfg.d, cfg.t, cfg.dh, cfg.hpc, cfg.s_past
    s, nd, nt, ns, npair, g = cfg.s, cfg.nd, cfg.nt, cfg.ns, cfg.npair, cfg.g
    nsb_past = sp // 512  # past s-blocks (512 wide)
    ns_past = sp // 128   # past s-tiles (128 wide)

    # ---- I/O ----
    xT = nc.dram_tensor("xT", [d, t], F32, kind="ExternalInput").ap()
    wq = nc.dram_tensor("wq", [d, g], F32, kind="ExternalInput").ap()
    wk = nc.dram_tensor("wk", [d, g], F32, kind="ExternalInput").ap()
    wv = nc.dram_tensor("wv", [d, g], F32, kind="ExternalInput").ap()
    pastKhi = nc.dram_tensor("pastKhi", [hpc, dh, sp], F16, kind="ExternalInput").ap()
    pastKlo = nc.dram_tensor("pastKlo", [hpc, dh, sp], F16, kind="ExternalInput").ap()
    pastV = nc.dram_tensor("pastV", [hpc, sp, dh], F32, kind="ExternalInput").ap()
    wo16 = nc.dram_tensor("wo16", [g, d], F16, kind="ExternalInput").ap()

    out_shard = nc.dram_tensor(
        "out_shard", [cfg.tshard, d], F32, kind="ExternalOutput"
    ).ap()
    k_new = nc.dram_tensor("k_new", [hpc, t, dh], F32, kind="ExternalOutput").ap()
    v_new = nc.dram_tensor("v_new", [hpc, t, dh], F32, kind="ExternalOutput").ap()

    replica_groups = [
        list(range(gs, gs + cfg.group)) for gs in range(0, cfg.n_cores, cfg.group)
    ]

    with tile.TileContext(nc) as tc:
        with tc.tile_pool(name="persist", bufs=1) as pp:
            id16 = pp.tile([128, 128], F16)
            make_identity(nc, id16)
            id32 = pp.tile([128, 128], F32)
            make_identity(nc, id32)

            # persistent activations (pair-block row layout:
            # head 2pr at rows 0:64, head 2pr+1 at rows 64:128)
            qhi = pp.tile([128, npair, t], F16)
            qlo = pp.tile([128, npair, t], F16)
            khi_n = pp.tile([128, npair, t], F16)
            klo_n = pp.tile([128, npair, t], F16)
            v16 = pp.tile([128, nt, g], F16)      # new-V, [t_local, tt, h*dh]
            valuesT = pp.tile([128, npair, t], F16)
            l_all = pp.tile([128, hpc, nt], F32)
            linv_all = pp.tile([128, hpc, nt], F32)

            # =========== Phase A: projections ===========
            with tc.tile_pool(name="proj_sb", bufs=1) as wpool, \
                 tc.tile_pool(name="proj_xt", bufs=1) as xtp, \
                 tc.tile_pool(name="proj_st", bufs=3) as stp, \
                 tc.tile_pool(name="proj_kf", bufs=1) as kfp, \
                 tc.tile_pool(name="proj_ps", bufs=1, space="PSUM") as pps:
                wq_sb = wpool.tile([128, nd, g], F32)
                nc.sync.dma_start(wq_sb, wq.rearrange("(nd p) g -> p nd g", p=128))
                wk_sb = wpool.tile([128, nd, g], F32)
                nc.sync.dma_start(wk_sb, wk.rearrange("(nd p) g -> p nd g", p=128))
                wv_sb = wpool.tile([128, nd, g], F32)
                nc.sync.dma_start(wv_sb, wv.rearrange("(nd p) g -> p nd g", p=128))
                kTn = kfp.tile([128, npair, t], F32)  # fp32 k^T for the cache

                th_w = min(1024, t)
                for th0, thw in _blocks(t, th_w):
                    xt = []
                    for di in range(nd):
                        xti = xtp.tile([128, th_w], F32, tag=f"xt{di}")
                        nc.sync.dma_start(
                            xti[:, 0:thw], xT[di * 128:(di + 1) * 128, th0:th0 + thw]
                        )
                        xt.append(xti)
                    # q^T / k^T projections (fp32), split into f16 hi+lo
                    for wsb, hi_t, lo_t, f32_t in (
                        (wq_sb, qhi, qlo, None),
                        (wk_sb, khi_n, klo_n, kTn),
                    ):
                        for pr in range(npair):
                            m = 128 if 2 * pr + 1 < hpc else 64
                            for tb0, tbw in _blocks(thw, 512):
                                ps = pps.tile([128, 512], F32, tag="projps", bufs=3)
                                for di in range(nd):
                                    nc.tensor.matmul(
                                        ps[0:m, 0:tbw],
                                        lhsT=wsb[:, di, pr * 128:pr * 128 + m],
                                        rhs=xt[di][:, tb0:tb0 + tbw],
                                        start=(di == 0),
                                        stop=(di == nd - 1),
                                    )
                                dst = (slice(0, m), pr,
                                       slice(th0 + tb0, th0 + tb0 + tbw))
                                nc.any.tensor_copy(hi_t[dst], ps[0:m, 0:tbw])
                                # lo = fp32 - hi (cast to f16)
                                nc.vector.scalar_tensor_tensor(
                                    lo_t[dst], ps[0:m, 0:tbw], 1.0, hi_t[dst],
                                    op0=ALU.bypass, op1=ALU.subtract,
                                )
                                if f32_t is not None:
                                    nc.any.tensor_copy(f32_t[dst], ps[0:m, 0:tbw])
                    # v projection (fp32 -> f16 for PV, fp32 for the cache)
                    for tb0, tbw in _blocks(thw, 128):
                        tt = (th0 + tb0) // 128
                        ps = pps.tile([128, g], F32, tag="projps_v", bufs=3)
                        for di in range(nd):
                            nc.tensor.matmul(
                                ps,
                                lhsT=xt[di][:, tb0:tb0 + 128],
                                rhs=wv_sb[:, di, :],
                                start=(di == 0),
                                stop=(di == nd - 1),
                            )
                        nc.any.tensor_copy(v16[:, tt, :], ps)
                        vst = stp.tile([128, g], F32, tag="vstage")
                        nc.any.tensor_copy(vst, ps)
                        nc.sync.dma_start(
                            v_new[:, tt * 128:(tt + 1) * 128, :].rearrange(
                                "h t e -> t h e"
                            ),
                            vst.rearrange("p (h e) -> p h e", e=dh),
                        )
                # k_new cache = transpose(kTn) (fp32 exact)
                for pr in range(npair):
                    m = 128 if 2 * pr + 1 < hpc else 64
                    hpr = m // 64
                    for tt in range(nt):
                        tp = pps.tile([128, 128], F32, tag="ktrans", bufs=2)
                        nc.tensor.transpose(
                            tp[:, 0:m],
                            in_=kTn[0:m, pr, tt * 128:(tt + 1) * 128],
                            identity=id32[0:m, 0:m],
                        )
                        kst = stp.tile([128, 128], F32, tag="kstage")
                        nc.any.tensor_copy(kst[:, 0:m], tp[:, 0:m])
                        nc.sync.dma_start(
                            k_new[
                                2 * pr:2 * pr + hpr, tt * 128:(tt + 1) * 128, :
                            ].rearrange("h t e -> t h e"),
                            kst[:, 0:m].rearrange("p (h e) -> p h e", e=dh),
                        )

            # =========== Phase B: attention ===========
            if "B" in cfg.phases:
              with tc.tile_pool(name="at_kv", bufs=2) as kvp, \
                 tc.tile_pool(name="at_attn", bufs=3) as ap_, \
                 tc.tile_pool(name="at_attnT", bufs=1) as atp, \
                 tc.tile_pool(name="at_stats", bufs=8) as stq, \
                 tc.tile_pool(name="at_scps", bufs=2, space="PSUM") as scps, \
                 tc.tile_pool(name="at_tpps", bufs=2, space="PSUM") as tpps, \
                 tc.tile_pool(name="at_pvps", bufs=2, space="PSUM") as pvps:
                for pr in range(npair):
                    hpr = 2 if 2 * pr + 1 < hpc else 1
                    kThp = kvp.tile([128, sp], F16, tag="kThp")
                    kTlp = kvp.tile([128, sp], F16, tag="kTlp")
                    pV = kvp.tile([128, ns_past, 128], F16, tag="pastv")
                    for hi in range(hpr):
                        nc.sync.dma_start(
                            kThp[64 * hi:64 * hi + 64, :], pastKhi[2 * pr + hi]
                        )
                        nc.sync.dma_start(
                            kTlp[64 * hi:64 * hi + 64, :], pastKlo[2 * pr + hi]
                        )
                        nc.gpsimd.dma_start(
                            pV[:, :, 64 * hi:64 * hi + 64],
                            pastV[2 * pr + hi].rearrange("(st p) e -> p st e", p=128),
                        )

                    attnT = None
                    ch_w = min(1024, s)
                    chunks = _blocks(s, ch_w)
                    nch = len(chunks)
                    for tt in range(nt):
                        attn_t = []
                        nm_parts = []
                        l_parts = []
                        for hi in range(hpr):
                            attn_t.append(
                                ap_.tile(
                                    [128, s], F16, tag=f"attn{hi}",
                                    name=f"attn{hi}_{pr}_{tt}",
                                )
                            )
                            nm_parts.append(
                                stq.tile(
                                    [128, nch], F32, tag=f"nm{hi}",
                                    name=f"nm{hi}_{pr}_{tt}",
                                )
                            )
                            l_parts.append(
                                stq.tile(
                                    [128, nch], F32, tag=f"lp{hi}",
                                    name=f"lp{hi}_{pr}_{tt}",
                                )
                            )
                        # scores: 3-pass f16 split (hh, hl, lh) accumulated
                        # in fp32 PSUM; per-chunk -max then exp from PSUM.
                        for ci, (c0, cw) in enumerate(chunks):
                            for hi in range(hpr):
                                base = 64 * hi
                                qh = qhi[base:base + 64, pr, tt * 128:(tt + 1) * 128]
                                ql = qlo[base:base + 64, pr, tt * 128:(tt + 1) * 128]
                                ps = scps.tile([128, ch_w], F32, tag="scps")
                                for sb0, sbw in _blocks(cw, 512):
                                    sb = (c0 + sb0) // 512
                                    if sb < nsb_past:
                                        sl = slice(c0 + sb0, c0 + sb0 + sbw)
                                        kh = kThp[base:base + 64, sl]
                                        kl = kTlp[base:base + 64, sl]
                                    else:
                                        o = c0 + sb0 - sp
                                        kh = khi_n[base:base + 64, pr, o:o + sbw]
                                        kl = klo_n[base:base + 64, pr, o:o + sbw]
                                    out_sl = ps[:, sb0:sb0 + sbw]
                                    for pi, (lh_, rh_) in enumerate(
                                        ((qh, kh), (qh, kl), (ql, kh))
                                    ):
                                        nc.tensor.matmul(
                                            out_sl,
                                            lhsT=lh_,
                                            rhs=rh_,
                                            start=(pi == 0),
                                            stop=(pi == 2),
                                            tile_position=(base, 0),
                                        )
                                nc.vector.tensor_reduce(
                                    nm_parts[hi][:, ci:ci + 1],
                                    ps[:, 0:cw],
                                    axis=AX.X,
                                    op=ALU.max,
                                    negate=True,
                                )
                                nc.scalar.activation(
                                    attn_t[hi][:, c0:c0 + cw],
                                    ps[:, 0:cw],
                                    ACTF.Exp,
                                    bias=nm_parts[hi][:, ci:ci + 1],
                                    scale=1.0,
                                    accum_out=l_parts[hi][:, ci:ci + 1],
                                )
                        if tt % 2 == 0:
                            attnT = atp.tile(
                                [128, ns, 512], F16, tag="attnTj",
                                name=f"attnTj_{pr}_{tt}",
                            )
                        for hi in range(hpr):
                            h = 2 * pr + hi
                            attn = attn_t[hi]
                            # combine chunk stats:
                            #   m_final = -min_c nm_c ; f_c = exp(m_c - m_final)
                            #   l = sum_c l_c f_c ;     chunk scale = f_c / l
                            nmmin = stq.tile(
                                [128, 1], F32, tag=f"nmm{hi}",
                                name=f"nmm{hi}_{pr}_{tt}",
                            )
                            nc.vector.tensor_reduce(
                                nmmin, nm_parts[hi], axis=AX.X, op=ALU.min
                            )
                            f_parts = stq.tile(
                                [128, nch], F32, tag=f"fp{hi}",
                                name=f"fp{hi}_{pr}_{tt}",
                            )
                            nc.scalar.activation(
                                f_parts, nm_parts[hi], ACTF.Exp,
                                bias=nmmin, scale=-1.0,
                            )
                            lw = stq.tile(
                                [128, nch], F32, tag=f"lw{hi}",
                                name=f"lw{hi}_{pr}_{tt}",
                            )
                            nc.vector.tensor_mul(lw, l_parts[hi], f_parts)
                            nc.vector.reduce_sum(
                                l_all[:, h, tt:tt + 1], lw, axis=AX.X
                            )
                            nc.vector.reciprocal(
                                linv_all[:, h, tt:tt + 1], l_all[:, h, tt:tt + 1]
                            )
                            sc_parts = stq.tile(
                                [128, nch], F32, tag=f"scp{hi}",
                                name=f"scp{hi}_{pr}_{tt}",
                            )
                            nc.vector.tensor_scalar_mul(
                                sc_parts, f_parts, linv_all[:, h, tt:tt + 1]
                            )
                            for ci, (c0, cw) in enumerate(chunks):
                                nc.vector.tensor_scalar_mul(
                                    attn[:, c0:c0 + cw],
                                    attn[:, c0:c0 + cw],
                                    sc_parts[:, ci:ci + 1],
                                )
                            # transpose attn into joint [s, A|B] staging
                            for u in range(ns // 4):
                                tp = tpps.tile([128, 512], F16, tag="tp")
                                for j in range(4):
                                    st = 4 * u + j
                                    nc.tensor.transpose(
                                        tp[:, j * 128:(j + 1) * 128],
                                        in_=attn[:, st * 128:(st + 1) * 128],
                                        identity=id16,
                                    )
                                co = 256 * hi + 128 * (tt % 2)
                                nc.any.tensor_copy(
                                    attnT[:, 4 * u:4 * u + 4, co:co + 128],
                                    tp.rearrange("p (j c) -> p j c", j=4),
                                )
                        if tt % 2 == 1:
                            # A@V: one 128-col (FWL) V load + one N=512 MM
                            # per s-tile; A results in rows 0:64 cols 0:256,
                            # B in rows 64:128 cols 256:512.
                            nw = 512 if hpr == 2 else 256
                            pv = pvps.tile([128, 512], F32, tag="pv")
                            for st in range(ns):
                                if st < ns_past:
                                    lhsT = pV[:, st, 0:64 * hpr]
                                else:
                                    c0v = 2 * pr * dh
                                    lhsT = v16[:, st - ns_past, c0v:c0v + 64 * hpr]
                                nc.tensor.matmul(
                                    pv[0:64 * hpr, 0:nw],
                                    lhsT=lhsT,
                                    rhs=attnT[:, st, 0:nw],
                                    start=(st == 0),
                                    stop=(st == ns - 1),
                                )
                            for hi in range(hpr):
                                nc.any.tensor_copy(
                                    valuesT[
                                        64 * hi:64 * hi + 64, pr,
                                        (tt - 1) * 128:(tt + 1) * 128,
                                    ],
                                    pv[64 * hi:64 * hi + 64,
                                       256 * hi:256 * hi + 256],
                                )

            # ==== Phase C: partial W_o + ReduceScatter ====
            if "C" in cfg.phases:
              with tc.tile_pool(name="wo_dram", bufs=1, space="DRAM") as drp, \
                 tc.tile_pool(name="wo_sb", bufs=1) as wop, \
                 tc.tile_pool(name="wo_st", bufs=4) as wst, \
                 tc.tile_pool(name="wo_ps", bufs=4, space="PSUM") as wps:
                bounce = drp.tile([t, d], F32)
                rs_out = drp.tile([cfg.tshard, d], F32)
                wo_sb = wop.tile([128, npair, d], F16)
                for pr in range(npair):
                    m = 128 if 2 * pr + 1 < hpc else 64
                    nc.sync.dma_start(
                        wo_sb[0:m, pr, :], wo16[pr * 128:pr * 128 + m, :]
                    )
                for tt in range(nt):
                    for nb0, nbw in _blocks(d, 512):
                        ps = wps.tile([128, 512], F32, tag="wops")
                        for pr in range(npair):
                            m = 128 if 2 * pr + 1 < hpc else 64
                            nc.tensor.matmul(
                                ps[:, 0:nbw],
                                lhsT=valuesT[0:m, pr, tt * 128:(tt + 1) * 128],
                                rhs=wo_sb[0:m, pr, nb0:nb0 + nbw],
                                start=(pr == 0),
                                stop=(pr == npair - 1),
                            )
                        ost = wst.tile([128, 512], F32, tag="ost")
                        nc.any.tensor_copy(ost[:, 0:nbw], ps[:, 0:nbw])
                        nc.sync.dma_start(
                            bounce[tt * 128:(tt + 1) * 128, nb0:nb0 + nbw],
                            ost[:, 0:nbw],
                        )
                nc.gpsimd.collective_compute(
                    "ReduceScatter",
                    ALU.add,
                    ins=[bounce.opt()],
                    outs=[rs_out.opt()],
                    replica_groups=replica_groups,
                )
                nc.sync.dma_start(out_shard, rs_out)

    nc.compile()
    return nc


# ---------------- host wrapper ----------------

_CACHE: dict = {}


def _get_program(cfg: Cfg):
    if cfg not in _CACHE:
        _CACHE[cfg] = build_program(cfg)
    return _CACHE[cfg]


def make_in_maps(cfg: Cfg, x, past_K, past_V, w_q, w_k, w_v, W_o):
    """Build the per-core input dicts from full inputs (numpy f32)."""
    hpc, group = cfg.hpc, cfg.group
    wq8 = (w_q / 8.0).astype(np.float32)
    wo16 = np.ascontiguousarray(W_o.astype(np.float16))
    in_maps = []
    for c in range(cfg.n_cores):
        b = c // group
        hs = hpc * (c % group)
        he = hs + hpc
        pkT = np.ascontiguousarray(past_K[b, hs:he].transpose(0, 2, 1))
        pkhi = pkT.astype(np.float16)
        pklo = (pkT - pkhi.astype(np.float32)).astype(np.float16)
        in_maps.append(
            {
                "xT": np.ascontiguousarray(x[b].T),
                "wq": np.ascontiguousarray(wq8[:, hs * cfg.dh:he * cfg.dh]),
                "wk": np.ascontiguousarray(w_k[:, hs * cfg.dh:he * cfg.dh]),
                "wv": np.ascontiguousarray(w_v[:, hs * cfg.dh:he * cfg.dh]),
                "pastKhi": pkhi,
                "pastKlo": pklo,
                "pastV": np.ascontiguousarray(past_V[b, hs:he]),
                "wo16": wo16[hs * cfg.dh:he * cfg.dh, :],
            }
        )
    return in_maps


def assemble_outputs(cfg: Cfg, results, past_K, past_V):
    hpc, group = cfg.hpc, cfg.group
    n_b = cfg.n_cores // group
    H = hpc * group
    out = np.empty((n_b, cfg.t, cfg.d), np.float32)
    K = np.empty((n_b, H, cfg.s, cfg.dh), np.float32)
    V = np.empty((n_b, H, cfg.s, cfg.dh), np.float32)
    K[:, :, : cfg.s_past] = past_K
    V[:, :, : cfg.s_past] = past_V
    for c in range(cfg.n_cores):
        b = c // group
        r = c % group
        hs = hpc * r
        he = hs + hpc
        out[b, r * cfg.tshard:(r + 1) * cfg.tshard] = results[c]["out_shard"]
        K[b, hs:he, cfg.s_past:] = results[c]["k_new"]
        V[b, hs:he, cfg.s_past:] = results[c]["v_new"]
    return out, K, V


def run(cfg: Cfg, x, past_K, past_V, w_q, w_k, w_v, W_o, **spmd_kwargs):
    nc = _get_program(cfg)
    in_maps = make_in_maps(cfg, x, past_K, past_V, w_q, w_k, w_v, W_o)
    res = run_bass_kernel_spmd(
        nc, in_maps, core_ids=list(range(cfg.n_cores)), **spmd_kwargs
    )
    out, K, V = assemble_outputs(cfg, res.results, past_K, past_V)
    return (out, K, V), res


def kernel(x, past_K, past_V, w_q, w_k, w_v, W_o):
    cfg = Cfg()
    args = [
        np.asarray(a, np.float32) for a in (x, past_K, past_V, w_q, w_k, w_v, W_o)
    ]
    (out, K, V), _ = run(cfg, *args)
    return out, K, V
